# revision 1
# baseline (speedup 1.0000x reference)
"""Inverted dot-product attention (softmax over the query axis) on 8 trn2 cores.

Shapes: query [8,16,1,64], key/value [8,65536,1,64].
reference returns (output [8,16,64], attn [8,1,16,65536]) where
  s[b,q,k]   = (q/8) . k
  attn       = softmax over q of s                  (per key column)
  norm[b,q]  = sum_k attn + eps
  output     = (attn/norm) @ v

Sharding: data-parallel over batch B=8, one batch per NeuronCore.

Per-core dataflow (keys on SBUF partitions, queries on the free axis):
  - host pre-transposes K to kT and stacks the two 32768-key halves on the
    partition axis -> kts [128, 32768]; host permutes V to v2 [128, 64, 8, 65]
    (last column of the 65 is ones, which makes the PV matmul accumulate the
    key-sum normalizer for free); q is pre-scaled by 1/8 and duplicated on
    both partition halves -> qt2 [128, 16].
  - device loop (16 iterations x 4096 keys):
      QK:   32 matmuls lhsT=kts-slice [64,128], rhs=qt2-half [64,16]
            -> s_psum [128, 32*16] (PSUM, one bank)
      softmax over q: exp on ScalarE (PSUM->SBUF, no max subtraction: scores
            are ~N(0,1), |s| < ~7, exp is safe in fp32), segmented
            reduce_sum over q -> den [128,32], reciprocal, broadcast multiply
      PV:   32 matmuls lhsT=v2-slice [128,65], rhs=attn-slice [128,16]
            accumulating outT_psum [65,16] over the whole kernel
            (row 64 = ones -> normalizer)
      attn out: 4x PE transpose [128,128] -> PSUM -> SBUF staging buffer,
            flushed as 1 MB DMAs every 4 iterations
  - endgame: transpose outT [65,16] -> [16,65], scale rows by 1/norm,
            DMA out [16,64].
"""

import numpy as np

import concourse.bass as bass
import concourse.mybir as mybir
import concourse.tile as tile
from concourse import bacc
from concourse.bass_utils import run_bass_kernel_spmd

F32 = mybir.dt.float32

B, Q, KTOT, D = 8, 16, 65536, 64
N_CORES = 8

# Device tiling parameters.
COLS = 2048                 # kts columns per hw iteration (= 4096 keys)
LG_PER_ITER = COLS // 512   # logical 512-column groups per iteration (4)
FLUSH_ITERS = 4             # hw iterations per attn-output flush window


def build_kernel(ktot: int = KTOT):
    """Build the per-core Bass program. All 8 cores run the same NEFF."""
    half = ktot // 2            # keys per partition-half
    n_iter = half // COLS       # hw iterations
    n_lg = half // 512          # total logical groups (64 for full size)
    attn_cols_per_s = ktot // 8  # contiguous attn columns per s-group (8192)

    nc = bacc.Bacc("TRN2", target_bir_lowering=False, debug=False)

    qt2_d = nc.dram_tensor("qt2", [128, Q], F32, kind="ExternalInput").ap()
    kts_d = nc.dram_tensor("kts", [128, half], F32, kind="ExternalInput").ap()
    v2_d = nc.dram_tensor("v2", [128, n_lg * 8 * (D + 1)], F32,
                          kind="ExternalInput").ap()
    id_d = nc.dram_tensor("ident", [128, 128], F32, kind="ExternalInput").ap()
    attn_d = nc.dram_tensor("attn", [Q, ktot], F32, kind="ExternalOutput").ap()
    out_d = nc.dram_tensor("out", [Q, D], F32, kind="ExternalOutput").ap()

    # attn viewed as [s, q, f]: column k = s*attn_cols_per_s + f
    attn_r = attn_d.rearrange("q (s f) -> s q f", s=8)

    with tile.TileContext(nc) as tc:
        with (
            tc.tile_pool(name="singles", bufs=1) as singles,
            tc.tile_pool(name="kt", bufs=3) as kpool,
            tc.tile_pool(name="vt", bufs=3) as vpool,
            tc.tile_pool(name="esb", bufs=2) as epool,
            tc.tile_pool(name="asb", bufs=2) as apool,
            tc.tile_pool(name="stats", bufs=2) as stats,
            tc.tile_pool(name="abuf", bufs=3) as abuf_pool,
            tc.tile_pool(name="spsum", bufs=2, space="PSUM") as spsum,
            tc.tile_pool(name="tpsum", bufs=2, space="PSUM") as tpsum,
            tc.tile_pool(name="opsum", bufs=1, space="PSUM") as opsum,
        ):
            qt_sb = singles.tile([128, Q], F32)
            nc.sync.dma_start(qt_sb[:, :], qt2_d)
            id_sb = singles.tile([128, 128], F32)
            nc.sync.dma_start(id_sb[:, :], id_d)

            outT = opsum.tile([D + 1, Q], F32)  # accumulates over all iters

            n_flush = (n_iter + FLUSH_ITERS - 1) // FLUSH_ITERS
            abuf = None

            for i in range(n_iter):
                kt = kpool.tile([128, COLS], F32)
                nc.sync.dma_start(kt[:, :], kts_d[:, i * COLS:(i + 1) * COLS])
                vt = vpool.tile([128, LG_PER_ITER, 8, D + 1], F32)
                vt_cols = LG_PER_ITER * 8 * (D + 1)
                nc.sync.dma_start(
                    vt[:, :, :, :],
                    v2_d[:, i * vt_cols:(i + 1) * vt_cols])

                # QK -> s_ps [128, LG*8*16]
                s_ps = spsum.tile([128, LG_PER_ITER * 128], F32)
                for lg in range(LG_PER_ITER):
                    for s in range(8):
                        hrow = 64 * (s // 4)
                        t = s % 4
                        g = lg * 8 + s
                        nc.tensor.matmul(
                            s_ps[:, g * 16:(g + 1) * 16],
                            kt[hrow:hrow + 64, lg * 512 + t * 128:
                               lg * 512 + (t + 1) * 128],
                            qt_sb[hrow:hrow + 64, :],
                            start=True, stop=True)

                # softmax over q (free axis), no max subtraction
                e_sb = epool.tile([128, LG_PER_ITER * 128], F32)
                nc.scalar.activation(e_sb[:, :], s_ps[:, :],
                                     mybir.ActivationFunctionType.Exp)
                den = stats.tile([128, LG_PER_ITER * 8], F32)
                nc.vector.reduce_sum(
                    den[:, :],
                    e_sb[:, :].rearrange("p (g q) -> p g q", q=Q),
                    axis=mybir.AxisListType.X)
                rden = stats.tile([128, LG_PER_ITER * 8], F32)
                nc.vector.reciprocal(rden[:, :], den[:, :])
                a_sb = apool.tile([128, LG_PER_ITER * 128], F32)
                nc.vector.tensor_tensor(
                    a_sb[:, :].rearrange("p (g q) -> p g q", q=Q),
                    e_sb[:, :].rearrange("p (g q) -> p g q", q=Q),
                    rden[:, :, None].broadcast_to([128, LG_PER_ITER * 8, Q]),
                    op=mybir.AluOpType.mult)

                # PV accumulate (row 64 of vt is ones -> normalizer row)
                for lg in range(LG_PER_ITER):
                    for s in range(8):
                        g = lg * 8 + s
                        nc.tensor.matmul(
                            outT[:, :],
                            vt[:, lg, s, :],
                            a_sb[:, g * 16:(g + 1) * 16],
                            start=(i == 0 and g == 0),
                            stop=(i == n_iter - 1 and g == LG_PER_ITER * 8 - 1))

                # attn transpose -> [s*16+q, key] layout, stage + flush
                if i % FLUSH_ITERS == 0:
                    abuf = abuf_pool.tile([128, FLUSH_ITERS * LG_PER_ITER * 128],
                                          F32)
                at_ps = tpsum.tile([128, LG_PER_ITER * 128], F32)
                for lg in range(LG_PER_ITER):
                    nc.tensor.transpose(
                        at_ps[:, lg * 128:(lg + 1) * 128],
                        a_sb[:, lg * 128:(lg + 1) * 128],
                        id_sb[:, :])
                off = (i % FLUSH_ITERS) * LG_PER_ITER * 128
                nc.scalar.copy(abuf[:, off:off + LG_PER_ITER * 128], at_ps[:, :])

                if i % FLUSH_ITERS == FLUSH_ITERS - 1 or i == n_iter - 1:
                    w = i // FLUSH_ITERS
                    wcols = FLUSH_ITERS * LG_PER_ITER * 128
                    f0 = w * wcols
                    ncols = off + LG_PER_ITER * 128
                    nc.sync.dma_start(
                        attn_r[:, :, f0:f0 + ncols], abuf[:, :ncols])

            # endgame: normalize output rows by the accumulated key-sums
            o_sb = singles.tile([D + 1, Q], F32)
            nc.scalar.copy(o_sb[:, :], outT[:, :])
            ot_ps = tpsum.tile([Q, D + 1], F32)
            nc.tensor.transpose(ot_ps[:, :], o_sb[:, :],
                                id_sb[0:D + 1, 0:D + 1])
            o2_sb = singles.tile([Q, D + 1], F32)
            nc.scalar.copy(o2_sb[:, :], ot_ps[:, :])
            rq = singles.tile([Q, 1], F32)
            nc.vector.reciprocal(rq[:, :], o2_sb[:, D:D + 1])
            of_sb = singles.tile([Q, D], F32)
            nc.vector.tensor_scalar_mul(of_sb[:, :], o2_sb[:, 0:D], rq[:, :])
            nc.sync.dma_start(out_d, of_sb[:, :])

    nc.compile()
    return nc


def prep_inputs(query: np.ndarray, key: np.ndarray, value: np.ndarray,
                ktot: int = KTOT):
    """Host-side per-core input layout. Returns list of 8 in_maps."""
    half = ktot // 2
    n_lg = half // 512
    scale = np.float32(1.0 / np.sqrt(np.float32(D)))
    ident = np.eye(128, dtype=np.float32)
    in_maps = []
    for b in range(B):
        qb = np.asarray(query[b, :, 0, :], dtype=np.float32) * scale  # [16,64]
        qT = np.ascontiguousarray(qb.T)                               # [64,16]
        qt2 = np.concatenate([qT, qT], axis=0)                        # [128,16]

        kb = np.asarray(key[b, :, 0, :], dtype=np.float32)            # [K,64]
        kr = kb.reshape(2, 4, n_lg, 128, D)      # [half, t, lg, p, d]
        kts = np.ascontiguousarray(kr.transpose(0, 4, 2, 1, 3)).reshape(
            128, half)                            # row=half*64+d, col=lg*512+t*128+p

        vb = np.asarray(value[b, :, 0, :], dtype=np.float32)
        vr = vb.reshape(2, 4, n_lg, 128, D)      # [half, t, lg, p, d]
        vperm = vr.transpose(3, 2, 0, 1, 4)      # [p, lg, half, t, d]
        vaug = np.concatenate(
            [vperm, np.ones((128, n_lg, 2, 4, 1), dtype=np.float32)], axis=-1)
        v2 = np.ascontiguousarray(vaug).reshape(128, n_lg * 8 * (D + 1))

        in_maps.append({"qt2": qt2, "kts": np.ascontiguousarray(kts),
                        "v2": v2, "ident": ident})
    return in_maps


_NC_CACHE = {}


def get_nc(ktot: int = KTOT):
    if ktot not in _NC_CACHE:
        _NC_CACHE[ktot] = build_kernel(ktot)
    return _NC_CACHE[ktot]


def run(query, key, value, trace: bool = False, ktot: int = KTOT):
    nc = get_nc(ktot)
    in_maps = prep_inputs(query, key, value, ktot)
    res = run_bass_kernel_spmd(nc, in_maps, core_ids=list(range(N_CORES)),
                               trace=trace)
    outs = np.stack([r["out"] for r in res.results])          # [8,16,64]
    attn = np.stack([r["attn"] for r in res.results])[:, None]  # [8,1,16,K]
    return (outs.astype(np.float32), attn.astype(np.float32)), res


def kernel(query, key, value):
    (outs, attn), _ = run(query, key, value)
    return (outs, attn)
